# revision 22
# baseline (speedup 1.0000x reference)
"""Trainium2 Bass kernel for nn_BoxesFromMasks — v6.

Per core (TL=2 frames), per frame of 8 chunks [128, 2048]:
  input:  seg shipped as int8 (ids < 64) — 4x less HBM/host traffic.
  encode: ACT affine (lo plane bitpattern) + GPSIMD affine (hi) then one ACT
          pass converts both f32-pattern planes to u32 one-hots in place.
  rows:   one grouped tensor_reduce (bitwise_or over X) per chunk gives the
          per-row 64-bit mask [2 planes] directly — no tree, no bulk fold.
  cols:   binary OR tree over the 8 chunks on DVE; final acc per frame is
          DMA-transposed as u16 and OR-folded across source partitions.
  extract: broadcast bit-test + fused (x&1)*value STT candidates +
          max-reduces; partition fold via u16 DMA transposes; col-slot
          permutation via 0/1 fp32 PE matmuls; empty-segment fixup in f32.
"""

import numpy as np

_T, _H, _W, _N = 16, 1024, 2048, 64
_NCORES = 8

_BUILD_CACHE = {}

BIG = 0x7FFF


def _make_tables(TL, H, W):
    P, CH, B = 128, H // 128, (2 * W) // 128
    pp = np.arange(P)

    yv = (np.arange(CH)[None, :] * P + pp[:, None]).astype(np.int64)  # [P, CH]
    yvt = np.stack([BIG - yv, yv + 1], axis=1).astype(np.int32)       # [P, 2, CH]

    bb = np.arange(B)
    xv = (64 * bb[None, :] + (pp[:, None] // 2)).astype(np.int64)     # [P, B]
    xvt = np.stack([BIG - xv, xv + 1], axis=1).astype(np.int16)       # [P, 2, B]

    bs32 = np.zeros((P, 2, 32), np.uint32)
    bs32[:, 0, :] = np.uint32(1) << (31 - np.arange(32, dtype=np.uint32))
    bs32[:, 1, :] = np.uint32(1) << np.arange(32, dtype=np.uint32)

    bs16 = np.broadcast_to(np.uint16(1) << np.arange(16, dtype=np.uint16),
                           (P, 16)).copy()

    permmin = np.zeros((2, P, P), np.float32)   # [h, src p, dst m]
    permmax = np.zeros((2, P, P), np.float32)
    for h in range(2):
        for pl in range(2):
            for j in range(16):
                s = (31 - (16 * h + j)) if pl == 0 else (32 + 16 * h + j)
                for f in range(TL):
                    src = pl * 32 + j * TL + f
                    dst = s * TL + f
                    permmin[h, src, dst] = 1.0
                    permmax[h, 64 + src, dst] = 1.0
    return {
        "yvt": yvt, "xvt": xvt, "bs32": bs32, "bs16": bs16,
                "permmin0": permmin[0], "permmin1": permmin[1],
        "permmax0": permmax[0], "permmax1": permmax[1],
    }


def _build_program(TL, H, W, split_waits=True, reps=1, gp_nodes=()):
    from contextlib import ExitStack

    import concourse.bass as bass
    import concourse.tile as tile
    import concourse.mybir as mybir
    from concourse.alu_op_type import AluOpType as Op

    f32 = mybir.dt.float32
    i32 = mybir.dt.int32
    u32 = mybir.dt.uint32
    u16 = mybir.dt.uint16
    i16 = mybir.dt.int16
    i8 = mybir.dt.int8
    Copy = mybir.ActivationFunctionType.Copy
    X = mybir.AxisListType.X

    P = 128
    CH = H // P
    B = (2 * W) // P
    tables = _make_tables(TL, H, W)

    nc = bass.Bass()
    seg_in = nc.dram_tensor("seg", [TL, H, W], i8, kind="ExternalInput")
    boxes_out = nc.dram_tensor("boxes", [TL, 64, 4], f32, kind="ExternalOutput")

    dts = {"yvt": i32, "xvt": i16, "bs32": u32, "bs16": u16,
           "permmin0": f32, "permmin1": f32, "permmax0": f32, "permmax1": f32}
    din = {n: nc.dram_tensor(n, list(tables[n].shape), dts[n],
                             kind="ExternalInput") for n in tables}

    with tile.TileContext(nc) as tc, ExitStack() as ctx:
        constp = ctx.enter_context(tc.tile_pool(name="consts", bufs=1))
        segp = ctx.enter_context(tc.tile_pool(name="segp", bufs=2))
        ep = ctx.enter_context(tc.tile_pool(name="ep", bufs=4))
        rtp = ctx.enter_context(tc.tile_pool(name="rtp", bufs=1))
        treep = ctx.enter_context(tc.tile_pool(name="treep", bufs=1))
        accTp = ctx.enter_context(tc.tile_pool(name="accTp", bufs=1))
        maskp = ctx.enter_context(tc.tile_pool(name="maskp", bufs=1))
        xp = ctx.enter_context(tc.tile_pool(name="xp", bufs=1))
        smallp = ctx.enter_context(tc.tile_pool(name="smallp", bufs=1))
        psump = ctx.enter_context(
            tc.tile_pool(name="psump", bufs=1, space=bass.MemorySpace.PSUM))

        s_pre = segp.tile([P, W], i8, tag="s", name="s_pre")
        for hh0 in range(2):
            cs0 = hh0 * (W // 2)
            nc.sync.dma_start(s_pre[:, cs0:cs0 + W // 2],
                              seg_in[0, 0:P, cs0:cs0 + W // 2])

        c = {}
        for n in tables:
            c[n] = constp.tile(list(tables[n].shape), dts[n], tag=f"c_{n}",
                               name=f"c_{n}")
            nc.sync.dma_start(c[n][:], din[n][:])

        for _rep in range(reps):
            rmask = maskp.tile([P, 2, TL, CH], u32, tag="rmask")
            cmS = maskp.tile([P, 2, TL, B], u16, tag="cmS")

            for f in range(TL):
                LA = treep.tile([P, 2, W], u32, tag="LA")
                LB = treep.tile([P, 2, W], u32, tag="LB")
                M0 = treep.tile([P, 2, W], u32, tag="M0")
                M1 = treep.tile([P, 2, W], u32, tag="M1")
                nodes = {}

                def tree_emit(name, dst, a, b):
                    eng = nc.gpsimd if name in gp_nodes else nc.vector
                    eng.tensor_tensor(dst, a, b, Op.bitwise_or)
                    nodes[name] = dst

                prev_u = None
                for ch in range(CH):
                    pre = _rep == 0 and f == 0 and ch == 0
                    s = s_pre if pre else segp.tile([P, W], i8)
                    e = ep.tile([P, 2, W], i32)
                    u = e[:].bitcast(u32)
                    split = f == 0 and ch < 2
                    if split:
                        hw_ = W // 2
                        for hh in range(2):
                            cs = hh * hw_
                            if not pre:
                                nc.sync.dma_start(
                                    s[:, cs:cs + hw_],
                                    seg_in[f, ch * P:(ch + 1) * P,
                                           cs:cs + hw_])
                            nc.scalar.activation(
                                e[:, 0, cs:cs + hw_], s[:, cs:cs + hw_],
                                Copy, bias=1325400064.0, scale=-8388608.0)
                            nc.gpsimd.tensor_scalar(
                                e[:, 1, cs:cs + hw_], s[:, cs:cs + hw_],
                                8388608, 796917760, Op.mult, Op.add)
                            nc.scalar.activation(
                                u[:, :, cs:cs + hw_],
                                e[:, :, cs:cs + hw_].bitcast(f32), Copy)
                        nc.vector.tensor_reduce(rmask[:, :, f, ch], u[:],
                                                axis=X, op=Op.bitwise_or)
                    else:
                        nc.sync.dma_start(s[0:64, :],
                                          seg_in[f, ch * P:ch * P + 64, :])
                        nc.sync.dma_start(s[64:128, :],
                                          seg_in[f, ch * P + 64:ch * P + 128, :])

                        nc.scalar.activation(e[:, 0, :], s[:], Copy,
                                             bias=1325400064.0,
                                             scale=-8388608.0)
                        nc.gpsimd.tensor_scalar(e[:, 1, :], s[:], 8388608,
                                                796917760, Op.mult, Op.add)
                        nc.scalar.activation(u, e[:].bitcast(f32), Copy)

                        nc.vector.tensor_reduce(rmask[:, :, f, ch], u[:],
                                                axis=X, op=Op.bitwise_or)

                    if ch % 2 == 0:
                        prev_u = u
                    else:
                        pair = ch // 2
                        leaf = ("L0", "L1", "L2", "L3")[pair]
                        dst = (LA, LB)[pair % 2][:]
                        tree_emit(leaf, dst, prev_u, u)
                        if pair == 1:
                            tree_emit("M0", M0[:], nodes["L0"], nodes["L1"])
                        elif pair == 3:
                            tree_emit("M1", M1[:], nodes["L2"], nodes["L3"])
                            nc.vector.tensor_tensor(M1[:], nodes["M0"],
                                                    nodes["M1"],
                                                    Op.bitwise_or)

                acc = M1

                accT = accTp.tile([P, 2, B, P], u16)
                for pl in range(2):
                    nc.sync.dma_start(accT[:, pl], acc[:, pl, :].bitcast(u16),
                                      transpose=True)
                w = 64
                while w >= 1:
                    nc.vector.tensor_tensor(accT[:, :, :, 0:w],
                                            accT[:, :, :, 0:w],
                                            accT[:, :, :, w:2 * w],
                                            Op.bitwise_or)
                    w //= 2
                nc.vector.tensor_copy(cmS[:, :, f, :], accT[:, :, :, 0])

            FC = TL * CH
            sh32 = xp.tile([P, 2, 32, FC], u32, tag="sh32")
            rm_b = (rmask[:].rearrange("p pl f c -> p pl (f c)")
                    .unsqueeze(2).broadcast_to((P, 2, 32, FC)))
            bs_b = c["bs32"][:].unsqueeze(3).broadcast_to((P, 2, 32, FC))
            nc.vector.tensor_tensor(sh32[:], rm_b, bs_b,
                                    Op.bitwise_and)

            candr = xp.tile([P, 2, 2, 32, TL, CH], i16, tag="candr")
            for k in range(2):
                vt = (c["yvt"][:, k, :].unsqueeze(1)
                      .broadcast_to((P, 2 * 32 * TL, CH)))
                nc.vector.scalar_tensor_tensor(
                    candr[:, k].rearrange("p pl sp f ch -> p (pl sp f) ch"),
                    sh32[:].rearrange("p pl sp (f ch) -> p (pl sp f) ch",
                                      ch=CH),
                    0.0, vt, Op.is_gt, Op.mult)
            crf = candr[:].rearrange("p k pl sp f ch -> p (k pl sp f) ch")
            w = CH // 2
            while w >= 1:
                nc.vector.tensor_tensor(crf[:, :, 0:w], crf[:, :, 0:w],
                                        crf[:, :, w:2 * w], Op.max)
                w //= 2
            rowred = candr[:, :, :, :, :, 0]

            FB = TL * B
            sh16 = xp.tile([P, 2, 16, FB], u16, tag="sh16")
            cm_b = (cmS[:].rearrange("p pl f b -> p pl (f b)")
                    .unsqueeze(2).broadcast_to((P, 2, 16, FB)))
            j_b = (c["bs16"][:].unsqueeze(1).unsqueeze(3)
                   .broadcast_to((P, 2, 16, FB)))
            nc.vector.tensor_tensor(sh16[:], cm_b, j_b,
                                    Op.bitwise_and)

            candc = xp.tile([P, 2, 2, 16, TL, B], i16, tag="candc")
            for k in range(2):
                vt = (c["xvt"][:, k, :].unsqueeze(1)
                      .broadcast_to((P, 2 * 16 * TL, B)))
                nc.vector.scalar_tensor_tensor(
                    candc[:, k].rearrange("p pl j f b -> p (pl j f) b"),
                    sh16[:].rearrange("p pl j (f b) -> p (pl j f) b", b=B),
                    0.0, vt, Op.is_gt, Op.mult)
            ccf = candc[:].rearrange("p k pl j f b -> p (k pl j f) b")
            w = B // 2
            while w >= 1:
                nc.vector.tensor_tensor(ccf[:, :, 0:w], ccf[:, :, 0:w],
                                        ccf[:, :, w:2 * w], Op.max)
                w //= 2
            colred = candc[:, :, :, :, :, 0]

            ST = smallp.tile([P, 3, P], i16)
            SFP = smallp.tile([P, 3, P], i16)
            nc.vector.tensor_copy(
                SFP[:, 0:2, :].rearrange("p a b -> p (a b)"),
                rowred.rearrange("p k pl sp f -> p (k pl sp f)"))
            nc.vector.tensor_copy(
                SFP[:, 2, :], colred.rearrange("p k pl j f -> p (k pl j f)"))
            for m in range(3):
                nc.sync.dma_start(ST[:, m, :], SFP[:, m, :], transpose=True)
            Rf = smallp.tile([P, 2], i16)
            nc.vector.tensor_reduce(Rf[:, 0].unsqueeze(1), ST[:, 0, :],
                                    axis=X, op=Op.max)
            nc.vector.tensor_reduce(Rf[:, 1].unsqueeze(1), ST[:, 1, :],
                                    axis=X, op=Op.max)
            Cf16 = smallp.tile([P, 2], i16)
            STpar = ST[:, 2].rearrange("p (q two) -> p two q", two=2)
            nc.vector.tensor_reduce(Cf16[:, 0].unsqueeze(1), STpar[:, 0, :],
                                    axis=X, op=Op.max)
            nc.vector.tensor_reduce(Cf16[:, 1].unsqueeze(1), STpar[:, 1, :],
                                    axis=X, op=Op.max)
            Cf = smallp.tile([P, 2], f32)
            nc.vector.tensor_copy(Cf[:], Cf16[:])

            psmin = psump.tile([P, 1], f32, tag="psmin")
            psmax = psump.tile([P, 1], f32, tag="psmax")
            nc.tensor.matmul(psmin[:], c["permmin0"][:], Cf[:, 0].unsqueeze(1),
                             start=True, stop=False)
            nc.tensor.matmul(psmin[:], c["permmin1"][:], Cf[:, 1].unsqueeze(1),
                             start=False, stop=True)
            nc.tensor.matmul(psmax[:], c["permmax0"][:], Cf[:, 0].unsqueeze(1),
                             start=True, stop=False)
            nc.tensor.matmul(psmax[:], c["permmax1"][:], Cf[:, 1].unsqueeze(1),
                             start=False, stop=True)

            BOF = smallp.tile([P, 4], f32)
            fix = smallp.tile([P, 4], f32)
            nc.vector.tensor_scalar(BOF[:, 0].unsqueeze(1), psmin[:], -1.0,
                                    float(BIG), Op.mult, Op.add)
            nc.vector.tensor_scalar(BOF[:, 2].unsqueeze(1), psmax[:], 1.0,
                                    -1.0, Op.mult, Op.add)
            rf_f = smallp.tile([P, 2], f32)
            nc.vector.tensor_copy(rf_f[:], Rf[:])
            nc.vector.tensor_scalar(BOF[:, 1].unsqueeze(1),
                                    rf_f[:, 0].unsqueeze(1), -1.0,
                                    float(BIG), Op.mult, Op.add)
            nc.vector.tensor_scalar(BOF[:, 3].unsqueeze(1),
                                    rf_f[:, 1].unsqueeze(1), 1.0,
                                    -1.0, Op.mult, Op.add)

            nc.vector.tensor_scalar(fix[:, 0:2], BOF[:, 0:2], 32767.0,
                                    2147450880.0, Op.is_equal, Op.mult)
            nc.vector.tensor_scalar(fix[:, 2:4], BOF[:, 2:4], -1.0,
                                    -2147483647.0, Op.is_equal, Op.mult)
            nc.vector.tensor_tensor(BOF[:], BOF[:], fix[:], Op.add)

            nc.sync.dma_start(boxes_out[:].transpose([1, 0, 2]), BOF[:])

    nc.finalize()
    if split_waits:
        _split_excess_waits(nc, mybir)
    return nc, tables


def _split_excess_waits(nc, mybir):
    n_split = 0
    for f in nc.m.functions:
        for bb in f.blocks:
            newl = []
            for ins in bb.instructions:
                si = ins.sync_info
                max_waits = 1
                if si and si.on_wait and len(si.on_wait) > max_waits:
                    waits = list(si.on_wait)
                    for j, w in enumerate(waits[max_waits:]):
                        nop = mybir.InstNoOp(
                            name=f"{ins.name}-w{j}", ins=[], outs=[],
                            engine=ins.engine,
                            sync_info=mybir.SyncInfo(on_wait=[w],
                                                     on_update=[]))
                        newl.append(nop)
                        n_split += 1
                    ins.sync_info = mybir.SyncInfo(on_wait=waits[:max_waits],
                                                   on_update=si.on_update)
                newl.append(ins)
            bb.instructions = newl
    return n_split


def _get_program(TL, H, W, reps=1):
    key = (TL, H, W, reps)
    if key not in _BUILD_CACHE:
        _BUILD_CACHE[key] = _build_program(TL, H, W, reps=reps)
    return _BUILD_CACHE[key]


def kernel(segmentation, num_instances=None, **_ignored):
    from concourse.bass_utils import run_bass_kernel_spmd

    seg = np.asarray(segmentation)
    T, H, W = seg.shape
    assert T % _NCORES == 0
    TL = T // _NCORES
    nc, tables = _get_program(TL, H, W)

    seg = np.ascontiguousarray(seg.astype(np.int8))
    in_maps = [{"seg": seg[i * TL:(i + 1) * TL], **tables}
               for i in range(_NCORES)]
    res = run_bass_kernel_spmd(nc, in_maps, list(range(_NCORES)))
    out = np.concatenate([res.results[i]["boxes"] for i in range(_NCORES)],
                         axis=0)
    return out.astype(np.float32)


# revision 33
# speedup vs baseline: 1.0064x; 1.0064x over previous
"""Trainium2 Bass kernel for nn_BoxesFromMasks — v6.

Per core (TL=2 frames), per frame of 8 chunks [128, 2048]:
  input:  seg shipped as int8 (ids < 64) — 4x less HBM/host traffic.
  encode: ACT affine (lo plane bitpattern) + GPSIMD affine (hi) then one ACT
          pass converts both f32-pattern planes to u32 one-hots in place.
  rows:   one grouped tensor_reduce (bitwise_or over X) per chunk gives the
          per-row 64-bit mask [2 planes] directly — no tree, no bulk fold.
  cols:   binary OR tree over the 8 chunks on DVE; final acc per frame is
          DMA-transposed as u16 and OR-folded across source partitions.
  extract: broadcast bit-test + fused (x&1)*value STT candidates +
          max-reduces; partition fold via u16 DMA transposes; col-slot
          permutation via 0/1 fp32 PE matmuls; empty-segment fixup in f32.
"""

import numpy as np

_T, _H, _W, _N = 16, 1024, 2048, 64
_NCORES = 8

_BUILD_CACHE = {}

BIG = 0x7FFF


def _make_tables(TL, H, W):
    P, CH, B = 128, H // 128, (2 * W) // 128
    pp = np.arange(P)

    yv = (np.arange(CH)[None, :] * P + pp[:, None]).astype(np.int64)  # [P, CH]
    yvt = np.stack([BIG - yv, yv + 1], axis=1).astype(np.int32)       # [P, 2, CH]

    bb = np.arange(B)
    xv = (64 * bb[None, :] + (pp[:, None] // 2)).astype(np.int64)     # [P, B]
    xvt = np.stack([BIG - xv, xv + 1], axis=1).astype(np.int16)       # [P, 2, B]

    bs32 = np.zeros((P, 2, 32), np.uint32)
    bs32[:, 0, :] = np.uint32(1) << (31 - np.arange(32, dtype=np.uint32))
    bs32[:, 1, :] = np.uint32(1) << np.arange(32, dtype=np.uint32)

    bs16 = np.broadcast_to(np.uint16(1) << np.arange(16, dtype=np.uint16),
                           (P, 16)).copy()

    permmin = np.zeros((2, P, P), np.float32)   # [h, src p, dst m]
    permmax = np.zeros((2, P, P), np.float32)
    for h in range(2):
        for pl in range(2):
            for j in range(16):
                s = (31 - (16 * h + j)) if pl == 0 else (32 + 16 * h + j)
                for f in range(TL):
                    src = pl * 32 + j * TL + f
                    dst = s * TL + f
                    permmin[h, src, dst] = 1.0
                    permmax[h, 64 + src, dst] = 1.0
    return {
        "yvt": yvt, "xvt": xvt, "bs32": bs32, "bs16": bs16,
                "permmin0": permmin[0], "permmin1": permmin[1],
        "permmax0": permmax[0], "permmax1": permmax[1],
    }


def _build_program(TL, H, W, split_waits=True, reps=1, gp_nodes=()):
    from contextlib import ExitStack

    import concourse.bass as bass
    import concourse.tile as tile
    import concourse.mybir as mybir
    from concourse.alu_op_type import AluOpType as Op

    f32 = mybir.dt.float32
    i32 = mybir.dt.int32
    u32 = mybir.dt.uint32
    u16 = mybir.dt.uint16
    i16 = mybir.dt.int16
    i8 = mybir.dt.int8
    Copy = mybir.ActivationFunctionType.Copy
    X = mybir.AxisListType.X

    P = 128
    CH = H // P
    B = (2 * W) // P
    tables = _make_tables(TL, H, W)

    nc = bass.Bass()
    seg_in = nc.dram_tensor("seg", [TL, H, W], i8, kind="ExternalInput")
    boxes_out = nc.dram_tensor("boxes", [TL, 64, 4], f32, kind="ExternalOutput")

    dts = {"yvt": i32, "xvt": i16, "bs32": u32, "bs16": u16,
           "permmin0": f32, "permmin1": f32, "permmax0": f32, "permmax1": f32}
    din = {n: nc.dram_tensor(n, list(tables[n].shape), dts[n],
                             kind="ExternalInput") for n in tables}

    with tile.TileContext(nc) as tc, ExitStack() as ctx:
        constp = ctx.enter_context(tc.tile_pool(name="consts", bufs=1))
        segp = ctx.enter_context(tc.tile_pool(name="segp", bufs=2))
        ep = ctx.enter_context(tc.tile_pool(name="ep", bufs=4))
        rtp = ctx.enter_context(tc.tile_pool(name="rtp", bufs=1))
        treep = ctx.enter_context(tc.tile_pool(name="treep", bufs=1))
        accTp = ctx.enter_context(tc.tile_pool(name="accTp", bufs=1))
        maskp = ctx.enter_context(tc.tile_pool(name="maskp", bufs=1))
        xp = ctx.enter_context(tc.tile_pool(name="xp", bufs=1))
        smallp = ctx.enter_context(tc.tile_pool(name="smallp", bufs=1))
        psump = ctx.enter_context(
            tc.tile_pool(name="psump", bufs=1, space=bass.MemorySpace.PSUM))

        s_pre = segp.tile([P, W], i8, tag="s", name="s_pre")
        for hh0 in range(2):
            cs0 = hh0 * (W // 2)
            nc.sync.dma_start(s_pre[:, cs0:cs0 + W // 2],
                              seg_in[0, 0:P, cs0:cs0 + W // 2])

        c = {}
        for n in tables:
            c[n] = constp.tile(list(tables[n].shape), dts[n], tag=f"c_{n}",
                               name=f"c_{n}")
            nc.sync.dma_start(c[n][:], din[n][:])

        for _rep in range(reps):
            rmask = maskp.tile([P, 2, TL, CH], u32, tag="rmask")
            cmS = maskp.tile([P, 2, TL, B], u16, tag="cmS")

            for f in range(TL):
                LA = treep.tile([P, 2, W], u32, tag="LA")
                LB = treep.tile([P, 2, W], u32, tag="LB")
                M0 = treep.tile([P, 2, W], u32, tag="M0")
                M1 = treep.tile([P, 2, W], u32, tag="M1")
                nodes = {}

                def tree_emit(name, dst, a, b):
                    eng = nc.gpsimd if name in gp_nodes else nc.vector
                    eng.tensor_tensor(dst, a, b, Op.bitwise_or)
                    nodes[name] = dst

                prev_u = None
                for ch in range(CH):
                    pre = _rep == 0 and f == 0 and ch == 0
                    s = s_pre if pre else segp.tile([P, W], i8)
                    e = ep.tile([P, 2, W], i32)
                    u = e[:].bitcast(u32)
                    split = f == 0 and ch < 2
                    if split:
                        # per-half reduces: DVE starts after half 0's encode
                        rm2 = rtp.tile([P, 2, 2], u32, tag="rm2")
                        hw_ = W // 2
                        for hh in range(2):
                            cs = hh * hw_
                            if not pre:
                                nc.sync.dma_start(
                                    s[:, cs:cs + hw_],
                                    seg_in[f, ch * P:(ch + 1) * P,
                                           cs:cs + hw_])
                            nc.scalar.activation(
                                e[:, 0, cs:cs + hw_], s[:, cs:cs + hw_],
                                Copy, bias=1325400064.0, scale=-8388608.0)
                            nc.gpsimd.tensor_scalar(
                                e[:, 1, cs:cs + hw_], s[:, cs:cs + hw_],
                                8388608, 796917760, Op.mult, Op.add)
                            nc.scalar.activation(
                                u[:, :, cs:cs + hw_],
                                e[:, :, cs:cs + hw_].bitcast(f32), Copy)
                            nc.vector.tensor_reduce(
                                rm2[:, :, hh], u[:, :, cs:cs + hw_],
                                axis=X, op=Op.bitwise_or)
                        nc.vector.tensor_tensor(rmask[:, :, f, ch],
                                                rm2[:, :, 0], rm2[:, :, 1],
                                                Op.bitwise_or)
                    else:
                        nc.sync.dma_start(s[0:64, :],
                                          seg_in[f, ch * P:ch * P + 64, :])
                        nc.sync.dma_start(s[64:128, :],
                                          seg_in[f, ch * P + 64:ch * P + 128, :])

                        nc.scalar.activation(e[:, 0, :], s[:], Copy,
                                             bias=1325400064.0,
                                             scale=-8388608.0)
                        nc.gpsimd.tensor_scalar(e[:, 1, :], s[:], 8388608,
                                                796917760, Op.mult, Op.add)
                        nc.scalar.activation(u, e[:].bitcast(f32), Copy)

                        nc.vector.tensor_reduce(rmask[:, :, f, ch], u[:],
                                                axis=X, op=Op.bitwise_or)

                    if ch % 2 == 0:
                        prev_u = u
                    else:
                        pair = ch // 2
                        leaf = ("L0", "L1", "L2", "L3")[pair]
                        dst = (LA, LB)[pair % 2][:]
                        tree_emit(leaf, dst, prev_u, u)
                        if pair == 1:
                            tree_emit("M0", M0[:], nodes["L0"], nodes["L1"])
                        elif pair == 3:
                            tree_emit("M1", M1[:], nodes["L2"], nodes["L3"])
                            nc.vector.tensor_tensor(M1[:], nodes["M0"],
                                                    nodes["M1"],
                                                    Op.bitwise_or)

                acc = M1
                accT = accTp.tile([P, 2, B, P], u16)
                for pl in range(2):
                    nc.sync.dma_start(accT[:, pl], acc[:, pl, :].bitcast(u16),
                                      transpose=True)
                w = 64
                while w >= 1:
                    nc.vector.tensor_tensor(accT[:, :, :, 0:w],
                                            accT[:, :, :, 0:w],
                                            accT[:, :, :, w:2 * w],
                                            Op.bitwise_or)
                    w //= 2
                nc.vector.tensor_copy(cmS[:, :, f, :], accT[:, :, :, 0])

            FC = TL * CH
            sh32 = xp.tile([P, 2, 32, FC], u32, tag="sh32")
            rm_b = (rmask[:].rearrange("p pl f c -> p pl (f c)")
                    .unsqueeze(2).broadcast_to((P, 2, 32, FC)))
            bs_b = c["bs32"][:].unsqueeze(3).broadcast_to((P, 2, 32, FC))
            nc.vector.tensor_tensor(sh32[:], rm_b, bs_b,
                                    Op.bitwise_and)

            candr = xp.tile([P, 2, 2, 32, TL, CH], i16, tag="candr")
            for k in range(2):
                vt = (c["yvt"][:, k, :].unsqueeze(1)
                      .broadcast_to((P, 2 * 32 * TL, CH)))
                nc.vector.scalar_tensor_tensor(
                    candr[:, k].rearrange("p pl sp f ch -> p (pl sp f) ch"),
                    sh32[:].rearrange("p pl sp (f ch) -> p (pl sp f) ch",
                                      ch=CH),
                    0.0, vt, Op.is_gt, Op.mult)
            crf = candr[:].rearrange("p k pl sp f ch -> p (k pl sp f) ch")
            w = CH // 2
            while w >= 1:
                nc.vector.tensor_tensor(crf[:, :, 0:w], crf[:, :, 0:w],
                                        crf[:, :, w:2 * w], Op.max)
                w //= 2
            rowred = candr[:, :, :, :, :, 0]

            # row-side partition fold launches now (overlaps col extraction)
            ST = smallp.tile([P, 3, P], i16)
            SFP = smallp.tile([P, 3, P], i16)
            nc.vector.tensor_copy(
                SFP[:, 0:2, :].rearrange("p a b -> p (a b)"),
                rowred.rearrange("p k pl sp f -> p (k pl sp f)"))
            for m in range(2):
                nc.sync.dma_start(ST[:, m, :], SFP[:, m, :], transpose=True)

            FB = TL * B
            sh16 = xp.tile([P, 2, 16, FB], u16, tag="sh16")
            cm_b = (cmS[:].rearrange("p pl f b -> p pl (f b)")
                    .unsqueeze(2).broadcast_to((P, 2, 16, FB)))
            j_b = (c["bs16"][:].unsqueeze(1).unsqueeze(3)
                   .broadcast_to((P, 2, 16, FB)))
            nc.vector.tensor_tensor(sh16[:], cm_b, j_b,
                                    Op.bitwise_and)

            candc = xp.tile([P, 2, 2, 16, TL, B], i16, tag="candc")
            for k in range(2):
                vt = (c["xvt"][:, k, :].unsqueeze(1)
                      .broadcast_to((P, 2 * 16 * TL, B)))
                nc.vector.scalar_tensor_tensor(
                    candc[:, k].rearrange("p pl j f b -> p (pl j f) b"),
                    sh16[:].rearrange("p pl j (f b) -> p (pl j f) b", b=B),
                    0.0, vt, Op.is_gt, Op.mult)
            ccf = candc[:].rearrange("p k pl j f b -> p (k pl j f) b")
            w = B // 2
            while w >= 1:
                nc.vector.tensor_tensor(ccf[:, :, 0:w], ccf[:, :, 0:w],
                                        ccf[:, :, w:2 * w], Op.max)
                w //= 2
            colred = candc[:, :, :, :, :, 0]

            nc.vector.tensor_copy(
                SFP[:, 2, :], colred.rearrange("p k pl j f -> p (k pl j f)"))
            nc.sync.dma_start(ST[:, 2, :], SFP[:, 2, :], transpose=True)
            # row-side finishers fill the ST[:,2] transpose latency (their
            # ST[:,0/1] inputs landed long ago)
            BOF = smallp.tile([P, 4], f32)
            Rf = smallp.tile([P, 2], i16)
            nc.vector.tensor_reduce(Rf[:, 0].unsqueeze(1), ST[:, 0, :],
                                    axis=X, op=Op.max)
            nc.vector.tensor_reduce(Rf[:, 1].unsqueeze(1), ST[:, 1, :],
                                    axis=X, op=Op.max)
            rf_f = smallp.tile([P, 2], f32)
            nc.vector.tensor_copy(rf_f[:], Rf[:])
            nc.vector.tensor_scalar(BOF[:, 1].unsqueeze(1),
                                    rf_f[:, 0].unsqueeze(1), -1.0,
                                    float(BIG), Op.mult, Op.add)
            nc.vector.tensor_scalar(BOF[:, 3].unsqueeze(1),
                                    rf_f[:, 1].unsqueeze(1), 1.0,
                                    -1.0, Op.mult, Op.add)
            Cf16 = smallp.tile([P, 2], i16)
            STpar = ST[:, 2].rearrange("p (q two) -> p two q", two=2)
            nc.vector.tensor_reduce(Cf16[:, 0].unsqueeze(1), STpar[:, 0, :],
                                    axis=X, op=Op.max)
            nc.vector.tensor_reduce(Cf16[:, 1].unsqueeze(1), STpar[:, 1, :],
                                    axis=X, op=Op.max)
            Cf = smallp.tile([P, 2], f32)
            nc.vector.tensor_copy(Cf[:], Cf16[:])

            psmin = psump.tile([P, 1], f32, tag="psmin")
            psmax = psump.tile([P, 1], f32, tag="psmax")
            nc.tensor.matmul(psmin[:], c["permmin0"][:], Cf[:, 0].unsqueeze(1),
                             start=True, stop=False)
            nc.tensor.matmul(psmin[:], c["permmin1"][:], Cf[:, 1].unsqueeze(1),
                             start=False, stop=True)
            nc.tensor.matmul(psmax[:], c["permmax0"][:], Cf[:, 0].unsqueeze(1),
                             start=True, stop=False)
            nc.tensor.matmul(psmax[:], c["permmax1"][:], Cf[:, 1].unsqueeze(1),
                             start=False, stop=True)

            fix = smallp.tile([P, 4], f32)
            nc.vector.tensor_scalar(BOF[:, 0].unsqueeze(1), psmin[:], -1.0,
                                    float(BIG), Op.mult, Op.add)
            nc.vector.tensor_scalar(BOF[:, 2].unsqueeze(1), psmax[:], 1.0,
                                    -1.0, Op.mult, Op.add)

            nc.vector.tensor_scalar(fix[:, 0:2], BOF[:, 0:2], 32767.0,
                                    2147450880.0, Op.is_equal, Op.mult)
            nc.vector.tensor_scalar(fix[:, 2:4], BOF[:, 2:4], -1.0,
                                    -2147483647.0, Op.is_equal, Op.mult)
            nc.vector.tensor_tensor(BOF[:], BOF[:], fix[:], Op.add)

            nc.sync.dma_start(boxes_out[:].transpose([1, 0, 2]), BOF[:])

    nc.finalize()
    if split_waits:
        _split_excess_waits(nc, mybir)
    return nc, tables


def _split_excess_waits(nc, mybir):
    n_split = 0
    for f in nc.m.functions:
        for bb in f.blocks:
            newl = []
            for ins in bb.instructions:
                si = ins.sync_info
                max_waits = 1
                if si and si.on_wait and len(si.on_wait) > max_waits:
                    waits = list(si.on_wait)
                    for j, w in enumerate(waits[max_waits:]):
                        nop = mybir.InstNoOp(
                            name=f"{ins.name}-w{j}", ins=[], outs=[],
                            engine=ins.engine,
                            sync_info=mybir.SyncInfo(on_wait=[w],
                                                     on_update=[]))
                        newl.append(nop)
                        n_split += 1
                    ins.sync_info = mybir.SyncInfo(on_wait=waits[:max_waits],
                                                   on_update=si.on_update)
                newl.append(ins)
            bb.instructions = newl
    return n_split


def _get_program(TL, H, W, reps=1):
    key = (TL, H, W, reps)
    if key not in _BUILD_CACHE:
        _BUILD_CACHE[key] = _build_program(TL, H, W, reps=reps)
    return _BUILD_CACHE[key]


def kernel(segmentation, num_instances=None, **_ignored):
    from concourse.bass_utils import run_bass_kernel_spmd

    seg = np.asarray(segmentation)
    T, H, W = seg.shape
    assert T % _NCORES == 0
    TL = T // _NCORES
    nc, tables = _get_program(TL, H, W)

    seg = np.ascontiguousarray(seg.astype(np.int8))
    in_maps = [{"seg": seg[i * TL:(i + 1) * TL], **tables}
               for i in range(_NCORES)]
    res = run_bass_kernel_spmd(nc, in_maps, list(range(_NCORES)))
    out = np.concatenate([res.results[i]["boxes"] for i in range(_NCORES)],
                         axis=0)
    return out.astype(np.float32)


# revision 34
# speedup vs baseline: 1.0173x; 1.0108x over previous
"""Trainium2 Bass kernel for nn_BoxesFromMasks — v6.

Per core (TL=2 frames), per frame of 8 chunks [128, 2048]:
  input:  seg shipped as int8 (ids < 64) — 4x less HBM/host traffic.
  encode: ACT affine (lo plane bitpattern) + GPSIMD affine (hi) then one ACT
          pass converts both f32-pattern planes to u32 one-hots in place.
  rows:   one grouped tensor_reduce (bitwise_or over X) per chunk gives the
          per-row 64-bit mask [2 planes] directly — no tree, no bulk fold.
  cols:   binary OR tree over the 8 chunks on DVE; final acc per frame is
          DMA-transposed as u16 and OR-folded across source partitions.
  extract: broadcast bit-test + fused (x&1)*value STT candidates +
          max-reduces; partition fold via u16 DMA transposes; col-slot
          permutation via 0/1 fp32 PE matmuls; empty-segment fixup in f32.
"""

import numpy as np

_T, _H, _W, _N = 16, 1024, 2048, 64
_NCORES = 8

_BUILD_CACHE = {}

BIG = 0x7FFF


def _make_tables(TL, H, W):
    P, CH, B = 128, H // 128, (2 * W) // 128
    pp = np.arange(P)

    yv = (np.arange(CH)[None, :] * P + pp[:, None]).astype(np.int64)  # [P, CH]
    yvt = np.stack([BIG - yv, yv + 1], axis=1).astype(np.int32)       # [P, 2, CH]

    bb = np.arange(B)
    xv = (64 * bb[None, :] + (pp[:, None] // 2)).astype(np.int64)     # [P, B]
    xvt = np.stack([BIG - xv, xv + 1], axis=1).astype(np.int16)       # [P, 2, B]

    bs32 = np.zeros((P, 2, 32), np.uint32)
    bs32[:, 0, :] = np.uint32(1) << (31 - np.arange(32, dtype=np.uint32))
    bs32[:, 1, :] = np.uint32(1) << np.arange(32, dtype=np.uint32)

    bs16 = np.broadcast_to(np.uint16(1) << np.arange(16, dtype=np.uint16),
                           (P, 16)).copy()

    permmin = np.zeros((2, P, P), np.float32)   # [h, src p, dst m]
    permmax = np.zeros((2, P, P), np.float32)
    for h in range(2):
        for pl in range(2):
            for j in range(16):
                s = (31 - (16 * h + j)) if pl == 0 else (32 + 16 * h + j)
                for f in range(TL):
                    src = pl * 32 + j * TL + f
                    dst = s * TL + f
                    permmin[h, src, dst] = 1.0
                    permmax[h, 64 + src, dst] = 1.0
    return {
        "yvt": yvt, "xvt": xvt, "bs32": bs32, "bs16": bs16,
                "permmin0": permmin[0], "permmin1": permmin[1],
        "permmax0": permmax[0], "permmax1": permmax[1],
    }


def _build_program(TL, H, W, split_waits=True, reps=1, gp_nodes=()):
    from contextlib import ExitStack

    import concourse.bass as bass
    import concourse.tile as tile
    import concourse.mybir as mybir
    from concourse.alu_op_type import AluOpType as Op

    f32 = mybir.dt.float32
    i32 = mybir.dt.int32
    u32 = mybir.dt.uint32
    u16 = mybir.dt.uint16
    i16 = mybir.dt.int16
    i8 = mybir.dt.int8
    Copy = mybir.ActivationFunctionType.Copy
    X = mybir.AxisListType.X

    P = 128
    CH = H // P
    B = (2 * W) // P
    tables = _make_tables(TL, H, W)

    nc = bass.Bass()
    seg_in = nc.dram_tensor("seg", [TL, H, W], i8, kind="ExternalInput")
    boxes_out = nc.dram_tensor("boxes", [TL, 64, 4], f32, kind="ExternalOutput")

    dts = {"yvt": i32, "xvt": i16, "bs32": u32, "bs16": u16,
           "permmin0": f32, "permmin1": f32, "permmax0": f32, "permmax1": f32}
    din = {n: nc.dram_tensor(n, list(tables[n].shape), dts[n],
                             kind="ExternalInput") for n in tables}

    with tile.TileContext(nc) as tc, ExitStack() as ctx:
        constp = ctx.enter_context(tc.tile_pool(name="consts", bufs=1))
        segp = ctx.enter_context(tc.tile_pool(name="segp", bufs=2))
        ep = ctx.enter_context(tc.tile_pool(name="ep", bufs=4))
        rtp = ctx.enter_context(tc.tile_pool(name="rtp", bufs=1))
        treep = ctx.enter_context(tc.tile_pool(name="treep", bufs=1))
        accTp = ctx.enter_context(tc.tile_pool(name="accTp", bufs=1))
        maskp = ctx.enter_context(tc.tile_pool(name="maskp", bufs=1))
        xp = ctx.enter_context(tc.tile_pool(name="xp", bufs=1))
        smallp = ctx.enter_context(tc.tile_pool(name="smallp", bufs=1))
        psump = ctx.enter_context(
            tc.tile_pool(name="psump", bufs=1, space=bass.MemorySpace.PSUM))

        s_pre = segp.tile([P, W], i8, tag="s", name="s_pre")
        for hh0 in range(2):
            cs0 = hh0 * (W // 2)
            nc.sync.dma_start(s_pre[:, cs0:cs0 + W // 2],
                              seg_in[0, 0:P, cs0:cs0 + W // 2])

        c = {}
        for n in tables:
            c[n] = constp.tile(list(tables[n].shape), dts[n], tag=f"c_{n}",
                               name=f"c_{n}")
            nc.sync.dma_start(c[n][:], din[n][:])

        for _rep in range(reps):
            rmask = maskp.tile([P, 2, TL, CH], u32, tag="rmask")
            cmS = maskp.tile([P, 2, TL, B], u16, tag="cmS")

            for f in range(TL):
                LA = treep.tile([P, 2, W], u32, tag="LA")
                LB = treep.tile([P, 2, W], u32, tag="LB")
                M0 = treep.tile([P, 2, W], u32, tag="M0")
                M1 = treep.tile([P, 2, W], u32, tag="M1")
                nodes = {}

                def tree_emit(name, dst, a, b):
                    eng = nc.gpsimd if name in gp_nodes else nc.vector
                    eng.tensor_tensor(dst, a, b, Op.bitwise_or)
                    nodes[name] = dst

                prev_u = None
                for ch in range(CH):
                    pre = _rep == 0 and f == 0 and ch == 0
                    s = s_pre if pre else segp.tile([P, W], i8)
                    e = ep.tile([P, 2, W], i32)
                    u = e[:].bitcast(u32)
                    split = f == 0 and ch < 2
                    if split:
                        # per-half reduces: DVE starts after half 0's encode
                        rm2 = rtp.tile([P, 2, 2], u32, tag="rm2")
                        hw_ = W // 2
                        for hh in range(2):
                            cs = hh * hw_
                            if not pre:
                                nc.sync.dma_start(
                                    s[:, cs:cs + hw_],
                                    seg_in[f, ch * P:(ch + 1) * P,
                                           cs:cs + hw_])
                            nc.scalar.activation(
                                e[:, 0, cs:cs + hw_], s[:, cs:cs + hw_],
                                Copy, bias=1325400064.0, scale=-8388608.0)
                            nc.gpsimd.tensor_scalar(
                                e[:, 1, cs:cs + hw_], s[:, cs:cs + hw_],
                                8388608, 796917760, Op.mult, Op.add)
                            nc.scalar.activation(
                                u[:, :, cs:cs + hw_],
                                e[:, :, cs:cs + hw_].bitcast(f32), Copy)
                            nc.vector.tensor_reduce(
                                rm2[:, :, hh], u[:, :, cs:cs + hw_],
                                axis=X, op=Op.bitwise_or)
                        nc.vector.tensor_tensor(rmask[:, :, f, ch],
                                                rm2[:, :, 0], rm2[:, :, 1],
                                                Op.bitwise_or)
                    else:
                        nc.sync.dma_start(s[0:64, :],
                                          seg_in[f, ch * P:ch * P + 64, :])
                        nc.sync.dma_start(s[64:128, :],
                                          seg_in[f, ch * P + 64:ch * P + 128, :])

                        nc.scalar.activation(e[:, 0, :], s[:], Copy,
                                             bias=1325400064.0,
                                             scale=-8388608.0)
                        nc.gpsimd.tensor_scalar(e[:, 1, :], s[:], 8388608,
                                                796917760, Op.mult, Op.add)
                        nc.scalar.activation(u, e[:].bitcast(f32), Copy)

                        nc.vector.tensor_reduce(rmask[:, :, f, ch], u[:],
                                                axis=X, op=Op.bitwise_or)

                    if ch % 2 == 0:
                        prev_u = u
                    else:
                        pair = ch // 2
                        leaf = ("L0", "L1", "L2", "L3")[pair]
                        dst = (LA, LB)[pair % 2][:]
                        tree_emit(leaf, dst, prev_u, u)
                        if pair == 1:
                            tree_emit("M0", M0[:], nodes["L0"], nodes["L1"])
                        elif pair == 3:
                            tree_emit("M1", M1[:], nodes["L2"], nodes["L3"])
                            nc.vector.tensor_tensor(M1[:], nodes["M0"],
                                                    nodes["M1"],
                                                    Op.bitwise_or)

                acc = M1
                accT = accTp.tile([P, 2, B, P], u16)
                for pl in range(2):
                    nc.sync.dma_start(accT[:, pl], acc[:, pl, :].bitcast(u16),
                                      transpose=True)
                # fold 128 source partitions: one grouped u32-view reduce
                # (lanewise OR of both u16 halves), then combine lo|hi
                accR = accTp.tile([P, 2, B], u32, tag="accR")
                nc.vector.tensor_reduce(
                    accR[:], accT[:].bitcast(u32), axis=X, op=Op.bitwise_or)
                aR16 = accR[:].bitcast(u16).rearrange(
                    "p pl (b two) -> p pl b two", two=2)
                nc.vector.tensor_tensor(cmS[:, :, f, :], aR16[:, :, :, 0],
                                        aR16[:, :, :, 1], Op.bitwise_or)

            FC = TL * CH
            sh32 = xp.tile([P, 2, 32, FC], u32, tag="sh32")
            rm_b = (rmask[:].rearrange("p pl f c -> p pl (f c)")
                    .unsqueeze(2).broadcast_to((P, 2, 32, FC)))
            bs_b = c["bs32"][:].unsqueeze(3).broadcast_to((P, 2, 32, FC))
            nc.vector.tensor_tensor(sh32[:], rm_b, bs_b,
                                    Op.bitwise_and)

            candr = xp.tile([P, 2, 2, 32, TL, CH], i16, tag="candr")
            for k in range(2):
                vt = (c["yvt"][:, k, :].unsqueeze(1)
                      .broadcast_to((P, 2 * 32 * TL, CH)))
                nc.vector.scalar_tensor_tensor(
                    candr[:, k].rearrange("p pl sp f ch -> p (pl sp f) ch"),
                    sh32[:].rearrange("p pl sp (f ch) -> p (pl sp f) ch",
                                      ch=CH),
                    0.0, vt, Op.is_gt, Op.mult)
            crf = candr[:].rearrange("p k pl sp f ch -> p (k pl sp f) ch")
            w = CH // 2
            while w >= 1:
                nc.vector.tensor_tensor(crf[:, :, 0:w], crf[:, :, 0:w],
                                        crf[:, :, w:2 * w], Op.max)
                w //= 2
            rowred = candr[:, :, :, :, :, 0]

            # row-side partition fold launches now (overlaps col extraction)
            ST = smallp.tile([P, 3, P], i16)
            SFP = smallp.tile([P, 3, P], i16)
            nc.vector.tensor_copy(
                SFP[:, 0:2, :].rearrange("p a b -> p (a b)"),
                rowred.rearrange("p k pl sp f -> p (k pl sp f)"))
            for m in range(2):
                nc.sync.dma_start(ST[:, m, :], SFP[:, m, :], transpose=True)

            FB = TL * B
            sh16 = xp.tile([P, 2, 16, FB], u16, tag="sh16")
            cm_b = (cmS[:].rearrange("p pl f b -> p pl (f b)")
                    .unsqueeze(2).broadcast_to((P, 2, 16, FB)))
            j_b = (c["bs16"][:].unsqueeze(1).unsqueeze(3)
                   .broadcast_to((P, 2, 16, FB)))
            nc.vector.tensor_tensor(sh16[:], cm_b, j_b,
                                    Op.bitwise_and)

            candc = xp.tile([P, 2, 2, 16, TL, B], i16, tag="candc")
            for k in range(2):
                vt = (c["xvt"][:, k, :].unsqueeze(1)
                      .broadcast_to((P, 2 * 16 * TL, B)))
                nc.vector.scalar_tensor_tensor(
                    candc[:, k].rearrange("p pl j f b -> p (pl j f) b"),
                    sh16[:].rearrange("p pl j (f b) -> p (pl j f) b", b=B),
                    0.0, vt, Op.is_gt, Op.mult)
            ccf = candc[:].rearrange("p k pl j f b -> p (k pl j f) b")
            w = B // 2
            while w >= 1:
                nc.vector.tensor_tensor(ccf[:, :, 0:w], ccf[:, :, 0:w],
                                        ccf[:, :, w:2 * w], Op.max)
                w //= 2
            colred = candc[:, :, :, :, :, 0]

            nc.vector.tensor_copy(
                SFP[:, 2, :], colred.rearrange("p k pl j f -> p (k pl j f)"))
            nc.sync.dma_start(ST[:, 2, :], SFP[:, 2, :], transpose=True)
            # row-side finishers fill the ST[:,2] transpose latency (their
            # ST[:,0/1] inputs landed long ago)
            BOF = smallp.tile([P, 4], f32)
            Rf = smallp.tile([P, 2], i16)
            nc.vector.tensor_reduce(Rf[:, 0].unsqueeze(1), ST[:, 0, :],
                                    axis=X, op=Op.max)
            nc.vector.tensor_reduce(Rf[:, 1].unsqueeze(1), ST[:, 1, :],
                                    axis=X, op=Op.max)
            rf_f = smallp.tile([P, 2], f32)
            nc.vector.tensor_copy(rf_f[:], Rf[:])
            nc.vector.tensor_scalar(BOF[:, 1].unsqueeze(1),
                                    rf_f[:, 0].unsqueeze(1), -1.0,
                                    float(BIG), Op.mult, Op.add)
            nc.vector.tensor_scalar(BOF[:, 3].unsqueeze(1),
                                    rf_f[:, 1].unsqueeze(1), 1.0,
                                    -1.0, Op.mult, Op.add)
            Cf16 = smallp.tile([P, 2], i16)
            STpar = ST[:, 2].rearrange("p (q two) -> p two q", two=2)
            nc.vector.tensor_reduce(Cf16[:, 0].unsqueeze(1), STpar[:, 0, :],
                                    axis=X, op=Op.max)
            nc.vector.tensor_reduce(Cf16[:, 1].unsqueeze(1), STpar[:, 1, :],
                                    axis=X, op=Op.max)
            Cf = smallp.tile([P, 2], f32)
            nc.vector.tensor_copy(Cf[:], Cf16[:])

            psmin = psump.tile([P, 1], f32, tag="psmin")
            psmax = psump.tile([P, 1], f32, tag="psmax")
            nc.tensor.matmul(psmin[:], c["permmin0"][:], Cf[:, 0].unsqueeze(1),
                             start=True, stop=False)
            nc.tensor.matmul(psmin[:], c["permmin1"][:], Cf[:, 1].unsqueeze(1),
                             start=False, stop=True)
            nc.tensor.matmul(psmax[:], c["permmax0"][:], Cf[:, 0].unsqueeze(1),
                             start=True, stop=False)
            nc.tensor.matmul(psmax[:], c["permmax1"][:], Cf[:, 1].unsqueeze(1),
                             start=False, stop=True)

            fix = smallp.tile([P, 4], f32)
            nc.vector.tensor_scalar(BOF[:, 0].unsqueeze(1), psmin[:], -1.0,
                                    float(BIG), Op.mult, Op.add)
            nc.vector.tensor_scalar(BOF[:, 2].unsqueeze(1), psmax[:], 1.0,
                                    -1.0, Op.mult, Op.add)

            nc.vector.tensor_scalar(fix[:, 0:2], BOF[:, 0:2], 32767.0,
                                    2147450880.0, Op.is_equal, Op.mult)
            nc.vector.tensor_scalar(fix[:, 2:4], BOF[:, 2:4], -1.0,
                                    -2147483647.0, Op.is_equal, Op.mult)
            nc.vector.tensor_tensor(BOF[:], BOF[:], fix[:], Op.add)

            nc.sync.dma_start(boxes_out[:].transpose([1, 0, 2]), BOF[:])

    nc.finalize()
    if split_waits:
        _split_excess_waits(nc, mybir)
    return nc, tables


def _split_excess_waits(nc, mybir):
    n_split = 0
    for f in nc.m.functions:
        for bb in f.blocks:
            newl = []
            for ins in bb.instructions:
                si = ins.sync_info
                max_waits = 1
                if si and si.on_wait and len(si.on_wait) > max_waits:
                    waits = list(si.on_wait)
                    for j, w in enumerate(waits[max_waits:]):
                        nop = mybir.InstNoOp(
                            name=f"{ins.name}-w{j}", ins=[], outs=[],
                            engine=ins.engine,
                            sync_info=mybir.SyncInfo(on_wait=[w],
                                                     on_update=[]))
                        newl.append(nop)
                        n_split += 1
                    ins.sync_info = mybir.SyncInfo(on_wait=waits[:max_waits],
                                                   on_update=si.on_update)
                newl.append(ins)
            bb.instructions = newl
    return n_split


def _get_program(TL, H, W, reps=1):
    key = (TL, H, W, reps)
    if key not in _BUILD_CACHE:
        _BUILD_CACHE[key] = _build_program(TL, H, W, reps=reps)
    return _BUILD_CACHE[key]


def kernel(segmentation, num_instances=None, **_ignored):
    from concourse.bass_utils import run_bass_kernel_spmd

    seg = np.asarray(segmentation)
    T, H, W = seg.shape
    assert T % _NCORES == 0
    TL = T // _NCORES
    nc, tables = _get_program(TL, H, W)

    seg = np.ascontiguousarray(seg.astype(np.int8))
    in_maps = [{"seg": seg[i * TL:(i + 1) * TL], **tables}
               for i in range(_NCORES)]
    res = run_bass_kernel_spmd(nc, in_maps, list(range(_NCORES)))
    out = np.concatenate([res.results[i]["boxes"] for i in range(_NCORES)],
                         axis=0)
    return out.astype(np.float32)
